# revision 3
# baseline (speedup 1.0000x reference)
"""Causal self-attention (B=4, T=4096, D=H=1024, fp32) on 8 Trainium2 cores.

Sharding: 2 cores per batch element (core pair). The 32 row-tiles of 128 are
interleaved between the 2 cores (core `pair` p owns global tiles p, p+2, ...),
balancing causal attention work.

Projections: each core projects Q and K for its OWN 2048 rows; the K^T halves
are exchanged between the pair with 2-rank AllGather collectives (2 chunks of
2MB, issued as early as possible) — K projection runs first so the collectives
overlap the V projection + early attention. V is projected redundantly by both
cores (local rows from xlt, peer rows from xrt) — duplicating V costs ~55us of
PE but removes 4MB from the slow (~40GB/s, serialized) collective path.

K^T / V live in SBUF in LOCAL|REMOTE halves: local block i (global 2i+pair) at
position i, peer block i at position 16+i. With tile-interleaved ownership the
attention loop is pair-independent in ADDRESSES: for q-tile j it uses local
blocks 0..j (diagonal mask triu on block j — same for both pairs) and remote
blocks 0..j, with the pair asymmetry absorbed by a host-supplied mask (block
j remote is all-zeros for pair 0, all-ones for pair 1). Only the collective
readback needs the runtime pair: two cond-predicated DMAs per transfer.

Attention is computed in S^T layout (scores[k, q]) so the probabilities come
out of the exp already transposed for the PV matmul — zero PE transposes (all
x transposition is done on the host). Softmax row sums come from a ones-column
matmul accumulated alongside PV.

Numerics: bf16 matmuls with fp32 PSUM accumulation; softmax skips
max-subtraction (scores ~N(0,1) after the 1/32 scale); exp on ScalarE,
probabilities stored bf16, final normalization fp32.
"""

import numpy as np

B, T, D, H = 4, 4096, 1024, 1024
P = 128
NCORES = 8
GROUPS = [[0, 1], [2, 3], [4, 5], [6, 7]]
TH = T // 2       # rows owned per core (2048)
NLB = TH // P     # local blocks per core (16)

DEFAULT_CFG = dict(
    cc_pieces=[[0, 1], [2], [3]],   # K pieces per AllGather chunk
    xt_bufs=5,
    xtq_bufs=2, qt_bufs=1, wt_bufs=34, ob_bufs=2,
    ps_a_bufs=3, ps_o_bufs=2,
    pa_psk_bufs=2, pa_psv_bufs=2,
)


def _emit(ctx, tc, xlt, xrt, wq, wk, wv, masks, ones2, outp, cfg):
    import concourse.mybir as mybir
    from contextlib import ExitStack as _ES

    nc = tc.nc
    f32 = mybir.dt.float32
    bf16 = mybir.dt.bfloat16
    Copy = mybir.ActivationFunctionType.Copy
    Exp = mybir.ActivationFunctionType.Exp
    SCALE = 1.0 / 32.0  # 1/sqrt(H)

    cc_pieces = cfg["cc_pieces"]     # K pieces (512 rows each) per chunk
    n_cc = len(cc_pieces)
    piece_chunk = {}
    for c, ps in enumerate(cc_pieces):
        for k, p_ in enumerate(ps):
            piece_chunk[p_] = (c, k)

    const = ctx.enter_context(tc.tile_pool(name="const", bufs=1))
    persist = ctx.enter_context(tc.tile_pool(name="persist", bufs=1))
    dram = ctx.enter_context(tc.tile_pool(name="dram", bufs=1, space="DRAM"))

    mask_sb = const.tile([P, 256], bf16, tag="mask")
    nc.sync.dma_start(out=mask_sb, in_=masks)
    ones_sb = const.tile([P, 2], bf16, tag="ones")
    nc.sync.dma_start(out=ones_sb, in_=ones2)

    # K^T [h%128, h//128, col]: cols [0:TH] local blocks, [TH:2TH] remote.
    # V [t%128, blk, h]: blks [0:16] local, [16:32] remote.
    KT = persist.tile([P, 8, T], bf16, tag="KT")
    V = persist.tile([P, T // P, H], bf16, tag="V")

    cc_ins, cc_outs = [], []
    for c in range(n_cc):
        w = len(cc_pieces[c]) * 4096
        cc_ins.append(dram.tile([P, w], bf16,
                                tag=f"cc_in{c}", name=f"cc_in{c}"))
        cc_outs.append(dram.tile([2 * P, w], bf16,
                                 tag=f"cc_out{c}", name=f"cc_out{c}"))

    is_even = (nc.sync.partition_id() % 2) == 0

    def load_weight(wdram, wsb, eng=None):
        # DRAM [1024,1024] bf16 -> SBUF [128, 8, 1024] (d = dc*128 + p)
        eng = eng or nc.sync
        for dc in range(8):
            eng.dma_start(out=wsb[:, dc, :], in_=wdram[dc * P:(dc + 1) * P, :])

    # ---------------- Phase A: projections + pair AllGather of K -------------
    with _ES() as pa:
        wpool = pa.enter_context(tc.tile_pool(name="pa_w", bufs=1))
        xtp = pa.enter_context(tc.tile_pool(name="pa_xt", bufs=cfg["xt_bufs"]))
        psk = pa.enter_context(
            tc.tile_pool(name="pa_psk", bufs=cfg["pa_psk_bufs"], space="PSUM"))
        psv = pa.enter_context(
            tc.tile_pool(name="pa_psv", bufs=cfg["pa_psv_bufs"], space="PSUM"))

        def load_xt(src, t0):
            xt = xtp.tile([P, 8, 512], bf16, tag="xt")
            for dc in range(8):
                nc.sync.dma_start(
                    out=xt[:, dc, :],
                    in_=src[dc * P:(dc + 1) * P, t0:t0 + 512])
            return xt

        xt0 = load_xt(xlt, 0)  # first x tile before the weights
        wk_sb = wpool.tile([P, 8, 1024], bf16, tag="wk")
        wv_sb = wpool.tile([P, 8, 1024], bf16, tag="wv")
        # split the weight loads across both HWDGE queues
        for dc in range(8):
            eng = nc.sync if dc < 4 else nc.scalar
            eng.dma_start(out=wk_sb[:, dc, :], in_=wk[dc * P:(dc + 1) * P, :])
        load_weight(wv, wv_sb, eng=nc.scalar)

        # K projection for my 2048 rows, written straight into KT local half,
        # staged out to the collective as each chunk completes. The xlt
        # tiles are kept (xt_bufs >= 5) and reused by the V projection.
        xts = [xt0]
        for s in range(4):
            t0 = s * 512
            xt = xts[s]
            if s < 3:
                xts.append(load_xt(xlt, t0 + 512))
            for hc in range(8):
                kp = psk.tile([P, 512], f32, tag="kp")
                for dc in range(8):
                    nc.tensor.matmul(
                        kp, lhsT=wk_sb[:, dc, hc * P:(hc + 1) * P],
                        rhs=xt[:, dc, :], start=(dc == 0), stop=(dc == 7))
                nc.vector.tensor_copy(out=KT[:, hc, t0:t0 + 512], in_=kp)
            c, sp_ = piece_chunk[s]
            for hc in range(8):
                nc.sync.dma_start(
                    out=cc_ins[c][:, sp_ * 4096 + hc * 512:
                                  sp_ * 4096 + (hc + 1) * 512],
                    in_=KT[:, hc, t0:t0 + 512])
            if sp_ == len(cc_pieces[c]) - 1:
                nc.gpsimd.collective_compute(
                    "AllGather",
                    mybir.AluOpType.bypass,
                    replica_groups=GROUPS,
                    ins=[cc_ins[c].opt()],
                    outs=[cc_outs[c].opt()],
                )

        # V projection: local rows (reusing the K loop's x tiles) then peer
        # rows, straight into V.
        def v_piece(xt, blk0):
            for i in range(4):
                vp = psv.tile([P, 1024], f32, tag="vp")
                for dc in range(8):
                    for nb in range(2):
                        nc.tensor.matmul(
                            vp[:, nb * 512:(nb + 1) * 512],
                            lhsT=xt[:, dc, i * P:(i + 1) * P],
                            rhs=wv_sb[:, dc, nb * 512:(nb + 1) * 512],
                            start=(dc == 0), stop=(dc == 7))
                nc.vector.tensor_copy(out=V[:, blk0 + i, :], in_=vp)

        xr0 = load_xt(xrt, 0)  # prefetch first peer piece during V-local
        for s in range(4):
            v_piece(xts[s], s * 4)
        for s in range(4):
            xr = xr0 if s == 0 else load_xt(xrt, s * 512)
            v_piece(xr, NLB + s * 4)

    def load_back(c):
        """Readback of the PEER's chunk-c K pieces into KT's remote half.

        cc_out rows [0:128] hold rank 0 (even core), [128:256] rank 1. The
        peer's rows depend on this core's parity, so each transfer is emitted
        twice with complementary cond predicates (skipped DMAs still bump
        their semaphores, so dependency tracking stays sound).
        """
        for s, p_ in enumerate(cc_pieces[c]):
            col = TH + p_ * 512
            # one 3D DMA per (piece, variant): the cond evaluation costs
            # ~1-3us of sync-engine ALU work per DMA, so consolidating 8
            # per-hc transfers into one is what keeps the readback fast
            src_lo = cc_outs[c][P:2 * P, s * 4096:(s + 1) * 4096]
            src_hi = cc_outs[c][0:P, s * 4096:(s + 1) * 4096]
            nc.sync.dma_start(out=KT[:, :, col:col + 512], in_=src_lo,
                              cond=is_even)
            nc.sync.dma_start(out=KT[:, :, col:col + 512], in_=src_hi,
                              cond=(is_even == 0))

    # ---------------- Phase B: Q projection + S^T attention ------------------
    with _ES() as pb:
        ec = pb.enter_context
        wqp = ec(tc.tile_pool(name="pb_w", bufs=1))
        xtqp = ec(tc.tile_pool(name="pb_xtq", bufs=cfg["xtq_bufs"]))
        qtp = ec(tc.tile_pool(name="pb_qt", bufs=cfg["qt_bufs"]))
        wtp = ec(tc.tile_pool(name="pb_wt", bufs=cfg["wt_bufs"]))
        obp = ec(tc.tile_pool(name="pb_ob", bufs=cfg["ob_bufs"]))
        smp = ec(tc.tile_pool(name="pb_sm", bufs=2))
        ps_a = ec(tc.tile_pool(name="pb_psa", bufs=cfg["ps_a_bufs"], space="PSUM"))
        ps_o = ec(tc.tile_pool(name="pb_pso", bufs=cfg["ps_o_bufs"], space="PSUM"))
        ps_s = ec(tc.tile_pool(name="pb_pss", bufs=1, space="PSUM"))

        wq_sb = wqp.tile([P, 8, 1024], bf16, tag="wq")
        load_weight(wq, wq_sb, eng=nc.scalar)

        # first superchunk that needs collective chunk c: piece p covers
        # remote blocks 4p..4p+3, first used by superchunk p
        sc_of_chunk = {c: min(cc_pieces[c]) for c in range(n_cc)}

        for sc in range(4):
            # Q^T for this superchunk: [h%128, hc, 512]
            # scalar HWDGE queue: the sync queue may be backed up behind
            # collective readbacks, which must not starve the Q projection
            xtq = xtqp.tile([P, 8, 512], bf16, tag="xtq")
            for dc in range(8):
                nc.scalar.dma_start(
                    out=xtq[:, dc, :],
                    in_=xlt[dc * P:(dc + 1) * P, sc * 512:(sc + 1) * 512])
            for c in range(n_cc):
                if sc_of_chunk[c] == sc:
                    load_back(c)
            qt = qtp.tile([P, 8, 512], bf16, tag="qt")
            for hc in range(8):
                qp = ps_a.tile([P, 512], f32, tag="psa")
                for dc in range(8):
                    nc.tensor.matmul(
                        qp, lhsT=wq_sb[:, dc, hc * P:(hc + 1) * P],
                        rhs=xtq[:, dc, :], start=(dc == 0), stop=(dc == 7))
                nc.vector.tensor_copy(out=qt[:, hc, :], in_=qp)

            NL = 4 * sc + 4           # local/remote block count this superchunk
            sums = ps_s.tile([P, 8], f32, tag="sums")
            wtL, wtR = [], []

            def do_tile(o, sc=sc, sums=sums, wtL=wtL, wtR=wtR):
                """Mask diag blocks, then sums+PV accumulation for q-tile o."""
                j = 4 * sc + o
                qs = slice(o * P, (o + 1) * P)
                nc.vector.tensor_mul(
                    wtL[j][:, qs], wtL[j][:, qs], mask_sb[:, 0:P])
                nc.vector.tensor_mul(
                    wtR[j][:, qs], wtR[j][:, qs], mask_sb[:, P:2 * P])
                op = ps_o.tile([P, 1024], f32, tag="op")
                nslice = 2 * (j + 1)
                ns = 0
                for i in range(j + 1):
                    for wt_, vb in ((wtL, i), (wtR, NLB + i)):
                        sl = wt_[i][:, qs]
                        nc.tensor.matmul(
                            sums[:, 2 * o:2 * o + 2], lhsT=sl, rhs=ones_sb,
                            start=(ns == 0), stop=(ns == nslice - 1))
                        for nb in range(2):
                            nc.tensor.matmul(
                                op[:, nb * 512:(nb + 1) * 512], lhsT=sl,
                                rhs=V[:, vb, nb * 512:(nb + 1) * 512],
                                start=(ns == 0), stop=(ns == nslice - 1))
                        ns += 1
                rec = smp.tile([P, 1], f32, tag="rec")
                nc.vector.reciprocal(out=rec, in_=sums[:, 2 * o:2 * o + 1])
                ob = obp.tile([P, 1024], bf16, tag="ob")
                nc.scalar.activation(out=ob, in_=op, func=Copy, scale=rec)
                jj = 4 * sc + o
                # scalar HWDGE queue keeps outputs off the (waiting) sync queue
                nc.scalar.dma_start(out=outp[jj * P:(jj + 1) * P, :], in_=ob)

            for i in range(NL):
                rel = i - 4 * sc
                off = 0 if rel < 1 else P * rel
                for wt_, kcol in ((wtL, i * P), (wtR, TH + i * P)):
                    sp = ps_a.tile([P, 512], f32, tag="psa")
                    for hc in range(8):
                        nc.tensor.matmul(
                            sp[:, off:], lhsT=KT[:, hc, kcol:kcol + P],
                            rhs=qt[:, hc, off:], start=(hc == 0), stop=(hc == 7))
                    wt_kb = wtp.tile([P, 512], bf16, tag="wt")
                    nc.scalar.activation(out=wt_kb[:, off:], in_=sp[:, off:],
                                         func=Exp, scale=SCALE)
                    wt_.append(wt_kb)
                # tile o's PV is emitted one i-step late so its last exp
                # hides under the next step's score matmuls
                if rel >= 1:
                    do_tile(rel - 1)
            do_tile(3)


def build_module(cfg=None):
    from contextlib import ExitStack
    import concourse.tile as tile
    import concourse.mybir as mybir
    from concourse import bacc

    full_cfg = dict(DEFAULT_CFG)
    if cfg:
        full_cfg.update(cfg)
    cfg = full_cfg
    dt = mybir.dt
    nc = bacc.Bacc("TRN2", target_bir_lowering=False, debug=False,
                   num_devices=NCORES)
    xlt = nc.dram_tensor("xlt", [D, TH], dt.bfloat16, kind="ExternalInput").ap()
    xrt = nc.dram_tensor("xrt", [D, TH], dt.bfloat16, kind="ExternalInput").ap()
    wq = nc.dram_tensor("wq", [D, H], dt.bfloat16, kind="ExternalInput").ap()
    wk = nc.dram_tensor("wk", [D, H], dt.bfloat16, kind="ExternalInput").ap()
    wv = nc.dram_tensor("wv", [D, H], dt.bfloat16, kind="ExternalInput").ap()
    masks = nc.dram_tensor("masks", [P, 256], dt.bfloat16, kind="ExternalInput").ap()
    ones2 = nc.dram_tensor("ones2", [P, 2], dt.bfloat16, kind="ExternalInput").ap()
    outp = nc.dram_tensor("outp", [TH, H], dt.bfloat16, kind="ExternalOutput").ap()

    with tile.TileContext(nc) as tc:
        with ExitStack() as ctx:
            _emit(ctx, tc, xlt, xrt, wq, wk, wv, masks, ones2, outp, cfg)
    nc.compile()
    return nc


def host_inputs(x, Wq, Wk, Wv):
    """Build the per-core input maps for run_bass_kernel_spmd."""
    import ml_dtypes
    bf = ml_dtypes.bfloat16

    xb = np.asarray(x, np.float32).astype(bf)
    wqb = np.asarray(Wq, np.float32).astype(bf)
    wkb = np.asarray(Wk, np.float32).astype(bf)
    wvb = np.asarray(Wv, np.float32).astype(bf)

    tri = np.triu(np.ones((P, P), np.float32))  # keep k <= q
    m = [np.concatenate([tri, np.zeros((P, P), np.float32)], 1).astype(bf),
         np.concatenate([tri, np.ones((P, P), np.float32)], 1).astype(bf)]
    ones2 = np.ones((P, 2), np.float32).astype(bf)

    in_maps = []
    xTs = [np.ascontiguousarray(xb[b].T) for b in range(B)]  # [1024, 4096]
    gathers = []
    for pair in range(2):
        idx = np.concatenate(
            [np.arange((2 * j + pair) * P, (2 * j + pair + 1) * P)
             for j in range(NLB)])
        gathers.append(idx)
    for c in range(NCORES):
        b, pair = c // 2, c % 2
        xT = xTs[b]
        in_maps.append({
            "xlt": np.ascontiguousarray(xT[:, gathers[pair]]),
            "xrt": np.ascontiguousarray(xT[:, gathers[1 - pair]]),
            "wq": wqb, "wk": wkb, "wv": wvb,
            "masks": m[pair], "ones2": ones2,
        })
    return in_maps


def gather_output(results):
    out = np.empty((B, T, H), np.float32)
    for c in range(NCORES):
        b, pair = c // 2, c % 2
        r = np.asarray(results[c]["outp"], np.float32)
        for j in range(NLB):
            out[b, (2 * j + pair) * P:(2 * j + pair + 1) * P, :] = \
                r[j * P:(j + 1) * P, :]
    return out


_NC_CACHE = {}


def kernel(x, Wq, Wk, Wv):
    from concourse.bass_utils import run_bass_kernel_spmd

    x = np.asarray(x, dtype=np.float32)
    Wq = np.asarray(Wq, dtype=np.float32)
    Wk = np.asarray(Wk, dtype=np.float32)
    Wv = np.asarray(Wv, dtype=np.float32)

    if "nc" not in _NC_CACHE:
        _NC_CACHE["nc"] = build_module()
    nc = _NC_CACHE["nc"]

    in_maps = host_inputs(x, Wq, Wk, Wv)
    res = run_bass_kernel_spmd(nc, in_maps, core_ids=list(range(NCORES)))
    return gather_output(res.results)
